# revision 43
# baseline (speedup 1.0000x reference)
"""Masked ragged-sequence mean on 8 Trainium2 NeuronCores.

out[b, d] = sum_{t < length[b]} input[b, t, d] / length[b]

Strategy (data-parallel over batch, per the problem's independence):
  - Samples are sorted by length (desc) and dealt to the 8 cores in bands
    of 8, so core slot j holds band-j samples of similar length. One SPMD
    program is compiled per length profile.
  - Per slot the program reads only the band MINIMUM tile count m_j (no
    padding waste); each core's per-sample surplus tiles are packed into a
    shared fixed-size overflow region. Guaranteed tiles are folded to
    [128, 256] by in-place pairwise trees of wide DVE adds (fp32
    tensor_tensor = 1 elem/lane/cycle) and one PE matmul with a [128, 1]
    column of 1/len reduces partitions + scales into PSUM [1, 256].
    Overflow tiles are routed on PE only: each gets a host-built [128, 8]
    lhsT whose single nonzero column (1/len in the tile's slot position)
    accumulates it into the right row of a shared [8, 256] PSUM tile.
    The host adds the overflow rows to the slot results.
  - The host zero-pads sample tails, so no on-device masking anywhere.
"""

import numpy as np

N_CORES = 8
P = 128    # SBUF partition count / token tile
CH = 11    # token tiles per DMA chunk (~1.4 MiB)
PE_K = 2   # tiles per guaranteed chunk reduced directly on PE

_runner_cache: dict = {}


def _plan(lens):
    """Band assignment + guaranteed/overflow split.

    Returns (assign[core, slot], m[slot], K_o, ov_tiles) where ov_tiles[c]
    is a list of (slot, tile_start, tile_end) per core.
    """
    B = lens.shape[0]
    S = B // N_CORES
    tiles = (lens + P - 1) // P
    order = np.argsort(-lens, kind="stable")
    assign = np.empty((N_CORES, S), dtype=np.int64)
    cum_ov = np.zeros(N_CORES, dtype=np.int64)
    m = np.empty(S, dtype=np.int64)
    # greedy per band: biggest surplus sample -> least-overflow-loaded core
    for j in range(S):
        band = order[j * N_CORES : (j + 1) * N_CORES]
        m[j] = max(1, int(tiles[band].min()))
        free = list(range(N_CORES))
        for b in sorted(band, key=lambda b: -(tiles[b] - m[j])):
            c = min(free, key=lambda c: cum_ov[c])
            assign[c, j] = b
            cum_ov[c] += tiles[b] - m[j]
            free.remove(c)
    K_o = int(cum_ov.max())
    ov_tiles = []
    for c in range(N_CORES):
        lst = []
        for j in range(S):
            t = int(tiles[assign[c, j]])
            if t > m[j]:
                lst.append((j, int(m[j]), t))
        ov_tiles.append(lst)
    return assign, tuple(int(v) for v in m), K_o, ov_tiles


def _build_program(S: int, D: int, m: tuple, K_o: int):
    import concourse.mybir as mybir
    import concourse.tile as tile
    from concourse import bacc

    f32 = mybir.dt.float32
    G = sum(m)

    nc = bacc.Bacc(
        "TRN2",
        target_bir_lowering=False,
        debug=False,
        enable_asserts=False,
        num_devices=N_CORES,
    )

    x_d = nc.dram_tensor("x", [G * P, D], f32, kind="ExternalInput")
    w_d = nc.dram_tensor("w", [P, S], f32, kind="ExternalInput")
    o_d = nc.dram_tensor("o", [S, D], f32, kind="ExternalOutput")
    if K_o:
        xo_d = nc.dram_tensor("xo", [K_o * P, D], f32, kind="ExternalInput")
        wo_d = nc.dram_tensor("wo", [P, K_o, 8], f32, kind="ExternalInput")
        oo_d = nc.dram_tensor("oo", [8, D], f32, kind="ExternalOutput")

    with tile.TileContext(nc) as tc:
        with (
            tc.tile_pool(name="xp", bufs=6) as xpool,
            tc.tile_pool(name="wp", bufs=1) as wpool,
            tc.tile_pool(name="ac", bufs=3) as apool,
            tc.tile_pool(name="op", bufs=2) as opool,
            tc.tile_pool(name="pp", bufs=7, space="PSUM") as ppool,
            tc.tile_pool(name="ppo", bufs=1, space="PSUM") as ppool_o,
        ):
            w_tile = wpool.tile([P, S], f32)
            nc.sync.dma_start(w_tile[:], w_d.ap())

            # ---- overflow region: PE-routed via per-tile [128, 8] lhsT ----
            # Emitted AFTER slot 0 so the first guaranteed chunk's DMA (which
            # gates the DVE fold pipeline) isn't queued behind the 2 MiB
            # overflow transfer; PE has plenty of slack later in the stream.
            def emit_overflow():
                wo_tile = wpool.tile([P, K_o, 8], f32)
                nc.sync.dma_start(wo_tile[:], wo_d.ap())
                xo_v = xo_d.ap().rearrange("(p n) d -> p n d", p=P, n=K_o)
                psum_o = ppool_o.tile([8, D], f32)
                ko_chunks = [
                    (c0, min(K_o, c0 + CH)) for c0 in range(0, K_o, CH)
                ]
                done = 0
                for c0, c1 in ko_chunks:
                    xot = xpool.tile([P, CH, D], f32, tag="xov")
                    nc.sync.dma_start(xot[:, : c1 - c0, :], xo_v[:, c0:c1, :])
                    for k in range(c0, c1):
                        nc.tensor.matmul(
                            psum_o[:],
                            wo_tile[:, k, :],
                            xot[:, k - c0, :],
                            start=(done == 0),
                            stop=(done == K_o - 1),
                        )
                        done += 1
                oo_tile = opool.tile([8, D], f32)
                nc.scalar.copy(oo_tile[:], psum_o[:])
                nc.scalar.dma_start(oo_d.ap(), oo_tile[:])

            # ---- guaranteed slots: per-chunk DVE tree fold + one matmul ----
            # All input chunks ride the single sync HWDGE queue: measured
            # faster than splitting across scalar (whose DMA issues queue
            # behind PSUM-copy waits) or GpSimd SWDGE (~1 us/DMA overhead).
            x_ap = x_d.ap()
            off = 0
            for s in range(S):
                nt = m[s]
                w_col = w_tile[:, s : s + 1]
                x_v = x_ap[off * P : (off + nt) * P, :].rearrange(
                    "(p n) d -> p n d", p=P, n=nt
                )
                off += nt
                chunks = [(c0, min(nt, c0 + CH)) for c0 in range(0, nt, CH)]
                multi = len(chunks) > 1

                psum_t = ppool.tile([1, D], f32)
                n_mm = 1 + sum(
                    PE_K if (c1 - c0) > PE_K + 1 else 0 for c0, c1 in chunks
                )
                mm_done = 0

                def mm(rhs):
                    nonlocal mm_done
                    nc.tensor.matmul(
                        psum_t[:],
                        w_col,
                        rhs,
                        start=(mm_done == 0),
                        stop=(mm_done == n_mm - 1),
                    )
                    mm_done += 1

                acc = None
                for ci, (c0, c1) in enumerate(chunks):
                    cn = c1 - c0
                    xt = xpool.tile([P, CH, D], f32)
                    nc.sync.dma_start(xt[:, :cn, :], x_v[:, c0:c1, :])
                    pe_take = PE_K if cn > PE_K + 1 else 0
                    for k in range(cn - pe_take, cn):
                        mm(xt[:, k, :])
                    # in-place pairwise tree; odd leftovers fold into tile 0
                    w_ = cn - pe_take
                    stop_at = 2 if (multi and ci == 0 and w_ >= 2) else 1
                    while w_ > stop_at:
                        if w_ % 2:
                            nc.vector.tensor_add(
                                xt[:, 0, :], xt[:, 0, :], xt[:, w_ - 1, :]
                            )
                            w_ -= 1
                        h = w_ // 2
                        nc.vector.tensor_add(
                            xt[:, 0:h, :], xt[:, 0:h, :], xt[:, h : 2 * h, :]
                        )
                        w_ = h
                    if not multi:
                        mm(xt[:, 0, :])
                    elif ci == 0:
                        acc = apool.tile([P, D], f32)
                        if w_ == 2:
                            nc.vector.tensor_add(
                                acc[:], xt[:, 0, :], xt[:, 1, :]
                            )
                        else:
                            nc.vector.tensor_copy(acc[:], xt[:, 0, :])
                    else:
                        nc.vector.tensor_add(acc[:], acc[:], xt[:, 0, :])
                if multi:
                    mm(acc[:])

                o_tile = opool.tile([1, D], f32)
                nc.scalar.copy(o_tile[:], psum_t[:])
                nc.scalar.dma_start(o_d.ap()[s : s + 1, :], o_tile[:])

                if s == 0 and K_o:
                    emit_overflow()

    nc.compile()
    return nc


def _prepare(x, lens):
    """Pack per-core inputs. Returns (assign, key, in_maps, S)."""
    B, L, D = x.shape
    S = B // N_CORES
    assign, m, K_o, ov_tiles = _plan(lens)
    G = sum(m)
    inv = (1.0 / lens.astype(np.float64)).astype(np.float32)

    in_maps = []
    for c in range(N_CORES):
        xg = np.zeros((G * P, D), dtype=np.float32)
        off = 0
        for j in range(S):
            b = assign[c, j]
            l = int(lens[b])
            take = min(l, m[j] * P)
            xg[off * P : off * P + take] = x[b, :take]
            off += m[j]
        wc = np.broadcast_to(inv[assign[c]][None, :], (P, S))
        im = {"x": xg, "w": np.ascontiguousarray(wc)}
        if K_o:
            xo = np.zeros((K_o * P, D), dtype=np.float32)
            wo = np.zeros((P, K_o, 8), dtype=np.float32)
            ko = 0
            for j, t0, t1 in ov_tiles[c]:
                b = assign[c, j]
                l = int(lens[b])
                for t in range(t0, t1):
                    take = min(l, (t + 1) * P) - t * P
                    if take > 0:
                        xo[ko * P : ko * P + take] = x[b, t * P : t * P + take]
                    wo[:, ko, j] = inv[b]
                    ko += 1
            # device reads overflow tile n as rows {p*K_o + n}; transpose so
            # host tile n lands there with per-partition-contiguous DMA runs
            im["xo"] = np.ascontiguousarray(
                xo.reshape(K_o, P, D).transpose(1, 0, 2).reshape(K_o * P, D)
            )
            im["wo"] = wo
        in_maps.append(im)
    return assign, (S, L, D, m, K_o), in_maps


def kernel(input, length):
    from concourse.bass_interp import get_hw_module
    from concourse.bass_utils import run_bass_kernel_spmd

    x = np.asarray(input, dtype=np.float32)
    lens = np.asarray(length).astype(np.int64)
    B, L, D = x.shape
    assert B % N_CORES == 0 and L % P == 0
    S = B // N_CORES

    assign, key, in_maps, = _prepare(x, lens)
    m, K_o = key[3], key[4]

    runner = _runner_cache.get(key)
    if runner is None:
        nc = _build_program(S, D, m, K_o)
        nc.m = get_hw_module(nc.m)
        runner = nc
        _runner_cache[key] = runner

    res = run_bass_kernel_spmd(runner, in_maps, core_ids=list(range(N_CORES)))

    out = np.empty((B, D), dtype=np.float32)
    for c in range(N_CORES):
        o = res.results[c]["o"]
        if K_o:
            o = o + res.results[c]["oo"]
        out[assign[c]] = o
    return out


# revision 49
# speedup vs baseline: 1.0410x; 1.0410x over previous
"""Masked ragged-sequence mean on 8 Trainium2 NeuronCores.

out[b, d] = sum_{t < length[b]} input[b, t, d] / length[b]

Strategy (data-parallel over batch, per the problem's independence):
  - Samples are sorted by length (desc) and dealt to the 8 cores in bands
    of 8, so core slot j holds band-j samples of similar length. One SPMD
    program is compiled per length profile.
  - Per slot the program reads only the band MINIMUM tile count m_j (no
    padding waste); each core's per-sample surplus tiles are packed into a
    shared fixed-size overflow region. Guaranteed tiles are folded to
    [128, 256] by in-place pairwise trees of wide DVE adds (fp32
    tensor_tensor = 1 elem/lane/cycle) and one PE matmul with a [128, 1]
    column of 1/len reduces partitions + scales into PSUM [1, 256].
    Overflow tiles are routed on PE only: each gets a host-built [128, 8]
    lhsT whose single nonzero column (1/len in the tile's slot position)
    accumulates it into the right row of a shared [8, 256] PSUM tile.
    The host adds the overflow rows to the slot results.
  - The host zero-pads sample tails, so no on-device masking anywhere.
"""

import numpy as np

N_CORES = 8
P = 128    # SBUF partition count / token tile
CH = 11    # token tiles per DMA chunk (~1.4 MiB)
PE_K = 2   # tiles per guaranteed chunk reduced directly on PE

_runner_cache: dict = {}


def _plan(lens):
    """Band assignment + guaranteed/overflow split.

    Returns (assign[core, slot], m[slot], K_o, ov_tiles) where ov_tiles[c]
    is a list of (slot, tile_start, tile_end) per core.
    """
    B = lens.shape[0]
    S = B // N_CORES
    tiles = (lens + P - 1) // P
    order = np.argsort(-lens, kind="stable")
    assign = np.empty((N_CORES, S), dtype=np.int64)
    cum_ov = np.zeros(N_CORES, dtype=np.int64)
    m = np.empty(S, dtype=np.int64)
    # greedy per band: biggest surplus sample -> least-overflow-loaded core
    for j in range(S):
        band = order[j * N_CORES : (j + 1) * N_CORES]
        m[j] = max(1, int(tiles[band].min()))
        free = list(range(N_CORES))
        for b in sorted(band, key=lambda b: -(tiles[b] - m[j])):
            c = min(free, key=lambda c: cum_ov[c])
            assign[c, j] = b
            cum_ov[c] += tiles[b] - m[j]
            free.remove(c)
    K_o = int(cum_ov.max())
    ov_tiles = []
    for c in range(N_CORES):
        lst = []
        for j in range(S):
            t = int(tiles[assign[c, j]])
            if t > m[j]:
                lst.append((j, int(m[j]), t))
        ov_tiles.append(lst)
    return assign, tuple(int(v) for v in m), K_o, ov_tiles


def _build_program(S: int, D: int, m: tuple, K_o: int):
    import concourse.mybir as mybir
    import concourse.tile as tile
    from concourse import bacc

    f32 = mybir.dt.float32
    G = sum(m)

    nc = bacc.Bacc(
        "TRN2",
        target_bir_lowering=False,
        debug=False,
        enable_asserts=False,
        num_devices=N_CORES,
    )

    x_d = nc.dram_tensor("x", [G * P, D], f32, kind="ExternalInput")
    w_d = nc.dram_tensor("w", [P, S], f32, kind="ExternalInput")
    o_d = nc.dram_tensor("o", [S, D], f32, kind="ExternalOutput")
    if K_o:
        xo_d = nc.dram_tensor("xo", [K_o * P, D], f32, kind="ExternalInput")
        wo_d = nc.dram_tensor("wo", [P, K_o, 8], f32, kind="ExternalInput")
        oo_d = nc.dram_tensor("oo", [8, D], f32, kind="ExternalOutput")

    with tile.TileContext(nc) as tc:
        with (
            tc.tile_pool(name="xp", bufs=6) as xpool,
            tc.tile_pool(name="wp", bufs=1) as wpool,
            tc.tile_pool(name="ac", bufs=3) as apool,
            tc.tile_pool(name="op", bufs=2) as opool,
            tc.tile_pool(name="pp", bufs=7, space="PSUM") as ppool,
            tc.tile_pool(name="ppo", bufs=1, space="PSUM") as ppool_o,
        ):
            w_tile = wpool.tile([P, S], f32)
            nc.sync.dma_start(w_tile[:], w_d.ap())

            # ---- overflow region: PE-routed via per-tile [128, 8] lhsT ----
            # Emitted AFTER slot 0 so the first guaranteed chunk's DMA (which
            # gates the DVE fold pipeline) isn't queued behind the 2 MiB
            # overflow transfer; PE has plenty of slack later in the stream.
            def emit_overflow():
                wo_tile = wpool.tile([P, K_o, 8], f32)
                nc.sync.dma_start(wo_tile[:], wo_d.ap())
                xo_v = xo_d.ap().rearrange("(p n) d -> p n d", p=P, n=K_o)
                psum_o = ppool_o.tile([8, D], f32)
                ko_chunks = [
                    (c0, min(K_o, c0 + CH)) for c0 in range(0, K_o, CH)
                ]
                done = 0
                for c0, c1 in ko_chunks:
                    xot = xpool.tile([P, CH, D], f32, tag="xov")
                    nc.sync.dma_start(xot[:, : c1 - c0, :], xo_v[:, c0:c1, :])
                    for k in range(c0, c1):
                        nc.tensor.matmul(
                            psum_o[:],
                            wo_tile[:, k, :],
                            xot[:, k - c0, :],
                            start=(done == 0),
                            stop=(done == K_o - 1),
                        )
                        done += 1
                oo_tile = opool.tile([8, D], f32)
                nc.scalar.copy(oo_tile[:], psum_o[:])
                nc.scalar.dma_start(oo_d.ap(), oo_tile[:])

            # ---- guaranteed slots: per-chunk DVE tree fold + one matmul ----
            x_ap = x_d.ap()
            off = 0
            for s in range(S):
                nt = m[s]
                w_col = w_tile[:, s : s + 1]
                x_v = x_ap[off * P : (off + nt) * P, :].rearrange(
                    "(p n) d -> p n d", p=P, n=nt
                )
                off += nt
                chunks = [(c0, min(nt, c0 + CH)) for c0 in range(0, nt, CH)]
                multi = len(chunks) > 1

                psum_t = ppool.tile([1, D], f32)
                n_mm = 1 + sum(
                    PE_K if (c1 - c0) > PE_K + 1 else 0 for c0, c1 in chunks
                )
                mm_done = 0

                def mm(rhs):
                    nonlocal mm_done
                    nc.tensor.matmul(
                        psum_t[:],
                        w_col,
                        rhs,
                        start=(mm_done == 0),
                        stop=(mm_done == n_mm - 1),
                    )
                    mm_done += 1

                acc = None
                for ci, (c0, c1) in enumerate(chunks):
                    cn = c1 - c0
                    xt = xpool.tile([P, CH, D], f32)
                    nc.sync.dma_start(xt[:, :cn, :], x_v[:, c0:c1, :])
                    pe_take = PE_K if cn > PE_K + 1 else 0
                    for k in range(cn - pe_take, cn):
                        mm(xt[:, k, :])
                    # in-place pairwise tree; odd leftovers fold into tile 0
                    w_ = cn - pe_take
                    stop_at = 2 if (multi and ci == 0 and w_ >= 2) else 1
                    while w_ > stop_at:
                        if w_ % 2:
                            nc.vector.tensor_add(
                                xt[:, 0, :], xt[:, 0, :], xt[:, w_ - 1, :]
                            )
                            w_ -= 1
                        h = w_ // 2
                        nc.vector.tensor_add(
                            xt[:, 0:h, :], xt[:, 0:h, :], xt[:, h : 2 * h, :]
                        )
                        w_ = h
                    if not multi:
                        mm(xt[:, 0, :])
                    elif ci == 0:
                        acc = apool.tile([P, D], f32)
                        if w_ == 2:
                            nc.vector.tensor_add(
                                acc[:], xt[:, 0, :], xt[:, 1, :]
                            )
                        else:
                            nc.vector.tensor_copy(acc[:], xt[:, 0, :])
                    else:
                        nc.vector.tensor_add(acc[:], acc[:], xt[:, 0, :])
                if multi:
                    mm(acc[:])

                o_tile = opool.tile([1, D], f32)
                nc.scalar.copy(o_tile[:], psum_t[:])
                nc.scalar.dma_start(o_d.ap()[s : s + 1, :], o_tile[:])

                if s == 0 and K_o:
                    emit_overflow()

    nc.compile()
    return nc


def _prepare(x, lens):
    """Pack per-core inputs. Returns (assign, key, in_maps, S)."""
    B, L, D = x.shape
    S = B // N_CORES
    assign, m, K_o, ov_tiles = _plan(lens)
    G = sum(m)
    inv = (1.0 / lens.astype(np.float64)).astype(np.float32)

    in_maps = []
    for c in range(N_CORES):
        xg = np.zeros((G * P, D), dtype=np.float32)
        off = 0
        for j in range(S):
            b = assign[c, j]
            l = int(lens[b])
            take = min(l, m[j] * P)
            xg[off * P : off * P + take] = x[b, :take]
            off += m[j]
        wc = np.broadcast_to(inv[assign[c]][None, :], (P, S))
        im = {"x": xg, "w": np.ascontiguousarray(wc)}
        if K_o:
            xo = np.zeros((K_o * P, D), dtype=np.float32)
            wo = np.zeros((P, K_o, 8), dtype=np.float32)
            ko = 0
            for j, t0, t1 in ov_tiles[c]:
                b = assign[c, j]
                l = int(lens[b])
                for t in range(t0, t1):
                    take = min(l, (t + 1) * P) - t * P
                    if take > 0:
                        xo[ko * P : ko * P + take] = x[b, t * P : t * P + take]
                    wo[:, ko, j] = inv[b]
                    ko += 1
            # device reads overflow tile n as rows {p*K_o + n}; transpose so
            # host tile n lands there with per-partition-contiguous DMA runs
            im["xo"] = np.ascontiguousarray(
                xo.reshape(K_o, P, D).transpose(1, 0, 2).reshape(K_o * P, D)
            )
            im["wo"] = wo
        in_maps.append(im)
    return assign, (S, L, D, m, K_o), in_maps


def kernel(input, length):
    from concourse.bass_interp import get_hw_module
    from concourse.bass_utils import run_bass_kernel_spmd

    x = np.asarray(input, dtype=np.float32)
    lens = np.asarray(length).astype(np.int64)
    B, L, D = x.shape
    assert B % N_CORES == 0 and L % P == 0
    S = B // N_CORES

    assign, key, in_maps, = _prepare(x, lens)
    m, K_o = key[3], key[4]

    runner = _runner_cache.get(key)
    if runner is None:
        nc = _build_program(S, D, m, K_o)
        nc.m = get_hw_module(nc.m)
        runner = nc
        _runner_cache[key] = runner

    res = run_bass_kernel_spmd(runner, in_maps, core_ids=list(range(N_CORES)))

    out = np.empty((B, D), dtype=np.float32)
    for c in range(N_CORES):
        o = res.results[c]["o"]
        if K_o:
            o = o + res.results[c]["oo"]
        out[assign[c]] = o
    return out


# revision 55
# speedup vs baseline: 1.0501x; 1.0087x over previous
"""Masked ragged-sequence mean on 8 Trainium2 NeuronCores.

out[b, d] = sum_{t < length[b]} input[b, t, d] / length[b]

Strategy (data-parallel over batch, per the problem's independence):
  - Samples are sorted by length (desc) and dealt to the 8 cores in bands
    of 8, so core slot j holds band-j samples of similar length. One SPMD
    program is compiled per length profile.
  - Per slot the program reads only the band MINIMUM tile count m_j (no
    padding waste); each core's per-sample surplus tiles are packed into a
    shared fixed-size overflow region. Guaranteed tiles are folded to
    [128, 256] by in-place pairwise trees of wide DVE adds (fp32
    tensor_tensor = 1 elem/lane/cycle) and one PE matmul with a [128, 1]
    column of 1/len reduces partitions + scales into PSUM [1, 256].
    Overflow tiles are routed on PE only: each gets a host-built [128, 8]
    lhsT whose single nonzero column (1/len in the tile's slot position)
    accumulates it into the right row of a shared [8, 256] PSUM tile.
    The host adds the overflow rows to the slot results.
  - The host zero-pads sample tails, so no on-device masking anywhere.
"""

import numpy as np

N_CORES = 8
P = 128    # SBUF partition count / token tile
CH = 11    # token tiles per DMA chunk (~1.4 MiB)
PE_K = 2   # tiles per guaranteed chunk reduced directly on PE

_runner_cache: dict = {}


def _plan(lens):
    """Band assignment + guaranteed/overflow split.

    Returns (assign[core, slot], m[slot], K_o, ov_tiles) where ov_tiles[c]
    is a list of (slot, tile_start, tile_end) per core.
    """
    B = lens.shape[0]
    S = B // N_CORES
    tiles = (lens + P - 1) // P
    order = np.argsort(-lens, kind="stable")
    assign = np.empty((N_CORES, S), dtype=np.int64)
    cum_ov = np.zeros(N_CORES, dtype=np.int64)
    m = np.empty(S, dtype=np.int64)
    # greedy per band: biggest surplus sample -> least-overflow-loaded core
    for j in range(S):
        band = order[j * N_CORES : (j + 1) * N_CORES]
        m[j] = max(1, int(tiles[band].min()))
        free = list(range(N_CORES))
        for b in sorted(band, key=lambda b: -(tiles[b] - m[j])):
            c = min(free, key=lambda c: cum_ov[c])
            assign[c, j] = b
            cum_ov[c] += tiles[b] - m[j]
            free.remove(c)
    K_o = int(cum_ov.max())
    ov_tiles = []
    for c in range(N_CORES):
        lst = []
        for j in range(S):
            t = int(tiles[assign[c, j]])
            if t > m[j]:
                lst.append((j, int(m[j]), t))
        ov_tiles.append(lst)
    return assign, tuple(int(v) for v in m), K_o, ov_tiles


def _build_program(S: int, D: int, m: tuple, K_o: int):
    import concourse.mybir as mybir
    import concourse.tile as tile
    from concourse import bacc

    f32 = mybir.dt.float32
    G = sum(m)

    nc = bacc.Bacc(
        "TRN2",
        target_bir_lowering=False,
        debug=False,
        enable_asserts=False,
        num_devices=N_CORES,
    )

    x_d = nc.dram_tensor("x", [G * P, D], f32, kind="ExternalInput")
    w_d = nc.dram_tensor("w", [P, S], f32, kind="ExternalInput")
    o_d = nc.dram_tensor("o", [S, D], f32, kind="ExternalOutput")
    if K_o:
        xo_d = nc.dram_tensor("xo", [K_o * P, D], f32, kind="ExternalInput")
        wo_d = nc.dram_tensor("wo", [P, K_o, 8], f32, kind="ExternalInput")
        oo_d = nc.dram_tensor("oo", [8, D], f32, kind="ExternalOutput")

    with tile.TileContext(nc) as tc:
        with (
            tc.tile_pool(name="xp", bufs=6) as xpool,
            tc.tile_pool(name="wp", bufs=1) as wpool,
            tc.tile_pool(name="ac", bufs=3) as apool,
            tc.tile_pool(name="op", bufs=2) as opool,
            tc.tile_pool(name="pp", bufs=7, space="PSUM") as ppool,
            tc.tile_pool(name="ppo", bufs=1, space="PSUM") as ppool_o,
        ):
            w_tile = wpool.tile([P, S], f32)
            nc.sync.dma_start(w_tile[:], w_d.ap())

            # ---- overflow region: PE-routed via per-tile [128, 8] lhsT ----
            # Emitted AFTER slot 0 so the first guaranteed chunk's DMA (which
            # gates the DVE fold pipeline) isn't queued behind the 2 MiB
            # overflow transfer; PE has plenty of slack later in the stream.
            def emit_overflow():
                wo_tile = wpool.tile([P, K_o, 8], f32)
                nc.sync.dma_start(wo_tile[:], wo_d.ap())
                xo_v = xo_d.ap().rearrange("(p n) d -> p n d", p=P, n=K_o)
                psum_o = ppool_o.tile([8, D], f32)
                ko_chunks = [
                    (c0, min(K_o, c0 + CH)) for c0 in range(0, K_o, CH)
                ]
                done = 0
                for c0, c1 in ko_chunks:
                    xot = xpool.tile([P, CH, D], f32, tag="xov")
                    nc.sync.dma_start(xot[:, : c1 - c0, :], xo_v[:, c0:c1, :])
                    for k in range(c0, c1):
                        nc.tensor.matmul(
                            psum_o[:],
                            wo_tile[:, k, :],
                            xot[:, k - c0, :],
                            start=(done == 0),
                            stop=(done == K_o - 1),
                        )
                        done += 1
                oo_tile = opool.tile([8, D], f32)
                nc.scalar.copy(oo_tile[:], psum_o[:])
                nc.scalar.dma_start(oo_d.ap(), oo_tile[:])

            # ---- guaranteed slots: per-chunk DVE tree fold + one matmul ----
            x_ap = x_d.ap()
            off = 0
            for s in range(S):
                nt = m[s]
                w_col = w_tile[:, s : s + 1]
                x_v = x_ap[off * P : (off + nt) * P, :].rearrange(
                    "(p n) d -> p n d", p=P, n=nt
                )
                off += nt
                chunks = [(c0, min(nt, c0 + CH)) for c0 in range(0, nt, CH)]
                multi = len(chunks) > 1

                psum_t = ppool.tile([1, D], f32)
                n_mm = 1 + sum(
                    PE_K if (c1 - c0) > PE_K + 1 else 0 for c0, c1 in chunks
                )
                mm_done = 0

                def mm(rhs):
                    nonlocal mm_done
                    nc.tensor.matmul(
                        psum_t[:],
                        w_col,
                        rhs,
                        start=(mm_done == 0),
                        stop=(mm_done == n_mm - 1),
                    )
                    mm_done += 1

                acc = None
                for ci, (c0, c1) in enumerate(chunks):
                    cn = c1 - c0
                    xt = xpool.tile([P, CH, D], f32)
                    nc.sync.dma_start(xt[:, :cn, :], x_v[:, c0:c1, :])
                    pe_take = PE_K if cn > PE_K + 1 else 0
                    for k in range(cn - pe_take, cn):
                        mm(xt[:, k, :])
                    # in-place pairwise tree; odd leftovers fold into tile 0
                    w_ = cn - pe_take
                    stop_at = 2 if (multi and ci == 0 and w_ >= 2) else 1
                    while w_ > stop_at:
                        if w_ % 2:
                            nc.vector.tensor_add(
                                xt[:, 0, :], xt[:, 0, :], xt[:, w_ - 1, :]
                            )
                            w_ -= 1
                        h = w_ // 2
                        nc.vector.tensor_add(
                            xt[:, 0:h, :], xt[:, 0:h, :], xt[:, h : 2 * h, :]
                        )
                        w_ = h
                    if not multi:
                        mm(xt[:, 0, :])
                    elif ci == 0:
                        acc = apool.tile([P, D], f32)
                        if w_ == 2:
                            nc.vector.tensor_add(
                                acc[:], xt[:, 0, :], xt[:, 1, :]
                            )
                        else:
                            nc.vector.tensor_copy(acc[:], xt[:, 0, :])
                    else:
                        nc.vector.tensor_add(acc[:], acc[:], xt[:, 0, :])
                if multi:
                    mm(acc[:])

                o_tile = opool.tile([1, D], f32)
                nc.scalar.copy(o_tile[:], psum_t[:])
                nc.scalar.dma_start(o_d.ap()[s : s + 1, :], o_tile[:])

                if s == 0 and K_o:
                    emit_overflow()

    nc.compile()
    return nc


def _prepare(x, lens):
    """Pack per-core inputs. Returns (assign, key, in_maps, S)."""
    B, L, D = x.shape
    S = B // N_CORES
    assign, m, K_o, ov_tiles = _plan(lens)
    G = sum(m)
    inv = (1.0 / lens.astype(np.float64)).astype(np.float32)

    in_maps = []
    for c in range(N_CORES):
        xg = np.zeros((G * P, D), dtype=np.float32)
        off = 0
        for j in range(S):
            b = assign[c, j]
            l = int(lens[b])
            take = min(l, m[j] * P)
            xg[off * P : off * P + take] = x[b, :take]
            off += m[j]
        wc = np.broadcast_to(inv[assign[c]][None, :], (P, S))
        im = {"x": xg, "w": np.ascontiguousarray(wc)}
        if K_o:
            xo = np.zeros((K_o * P, D), dtype=np.float32)
            wo = np.zeros((P, K_o, 8), dtype=np.float32)
            ko = 0
            for j, t0, t1 in ov_tiles[c]:
                b = assign[c, j]
                l = int(lens[b])
                for t in range(t0, t1):
                    take = min(l, (t + 1) * P) - t * P
                    if take > 0:
                        xo[ko * P : ko * P + take] = x[b, t * P : t * P + take]
                    wo[:, ko, j] = inv[b]
                    ko += 1
            # device reads overflow tile n as rows {p*K_o + n}; transpose so
            # host tile n lands there with per-partition-contiguous DMA runs
            im["xo"] = np.ascontiguousarray(
                xo.reshape(K_o, P, D).transpose(1, 0, 2).reshape(K_o * P, D)
            )
            im["wo"] = wo
        in_maps.append(im)
    return assign, (S, L, D, m, K_o), in_maps


def kernel(input, length):
    from concourse.bass_interp import get_hw_module
    from concourse.bass_utils import run_bass_kernel_spmd

    x = np.asarray(input, dtype=np.float32)
    lens = np.asarray(length).astype(np.int64)
    B, L, D = x.shape
    assert B % N_CORES == 0 and L % P == 0
    S = B // N_CORES

    assign, key, in_maps, = _prepare(x, lens)
    m, K_o = key[3], key[4]

    runner = _runner_cache.get(key)
    if runner is None:
        nc = _build_program(S, D, m, K_o)
        nc.m = get_hw_module(nc.m)
        runner = nc
        _runner_cache[key] = runner

    res = run_bass_kernel_spmd(runner, in_maps, core_ids=list(range(N_CORES)))

    out = np.empty((B, D), dtype=np.float32)
    for c in range(N_CORES):
        o = res.results[c]["o"]
        if K_o:
            o = o + res.results[c]["oo"]
        out[assign[c]] = o
    return out


# revision 57
# speedup vs baseline: 1.0674x; 1.0165x over previous
"""Masked ragged-sequence mean on 8 Trainium2 NeuronCores.

out[b, d] = sum_{t < length[b]} input[b, t, d] / length[b]

Strategy (data-parallel over batch, per the problem's independence):
  - Samples are sorted by length (desc) and dealt to the 8 cores in bands
    of 8, so core slot j holds band-j samples of similar length. One SPMD
    program is compiled per length profile.
  - Per slot the program reads only the band MINIMUM tile count m_j (no
    padding waste); each core's per-sample surplus tiles are packed into a
    shared fixed-size overflow region. Guaranteed tiles are folded to
    [128, 256] by in-place pairwise trees of wide DVE adds (fp32
    tensor_tensor = 1 elem/lane/cycle) and one PE matmul with a [128, 1]
    column of 1/len reduces partitions + scales into PSUM [1, 256].
    Overflow tiles are routed on PE only: each gets a host-built [128, 8]
    lhsT whose single nonzero column (1/len in the tile's slot position)
    accumulates it into the right row of a shared [8, 256] PSUM tile.
    The host adds the overflow rows to the slot results.
  - The host zero-pads sample tails, so no on-device masking anywhere.
"""

import numpy as np

N_CORES = 8
P = 128    # SBUF partition count / token tile
CH = 11    # token tiles per DMA chunk (~1.4 MiB)
PE_K = 2   # tiles per guaranteed chunk reduced directly on PE

_runner_cache: dict = {}


def _plan(lens):
    """Band assignment + guaranteed/overflow split.

    Returns (assign[core, slot], m[slot], K_o, ov_tiles) where ov_tiles[c]
    is a list of (slot, tile_start, tile_end) per core.
    """
    B = lens.shape[0]
    S = B // N_CORES
    tiles = (lens + P - 1) // P
    order = np.argsort(-lens, kind="stable")
    assign = np.empty((N_CORES, S), dtype=np.int64)
    cum_ov = np.zeros(N_CORES, dtype=np.int64)
    m = np.empty(S, dtype=np.int64)
    # greedy per band: biggest surplus sample -> least-overflow-loaded core
    for j in range(S):
        band = order[j * N_CORES : (j + 1) * N_CORES]
        m[j] = max(1, int(tiles[band].min()))
        free = list(range(N_CORES))
        for b in sorted(band, key=lambda b: -(tiles[b] - m[j])):
            c = min(free, key=lambda c: cum_ov[c])
            assign[c, j] = b
            cum_ov[c] += tiles[b] - m[j]
            free.remove(c)
    K_o = int(cum_ov.max())
    ov_tiles = []
    for c in range(N_CORES):
        lst = []
        for j in range(S):
            t = int(tiles[assign[c, j]])
            if t > m[j]:
                lst.append((j, int(m[j]), t))
        ov_tiles.append(lst)
    return assign, tuple(int(v) for v in m), K_o, ov_tiles


def _build_program(S: int, D: int, m: tuple, K_o: int):
    import concourse.mybir as mybir
    import concourse.tile as tile
    from concourse import bacc

    f32 = mybir.dt.float32
    G = sum(m)

    nc = bacc.Bacc(
        "TRN2",
        target_bir_lowering=False,
        debug=False,
        enable_asserts=False,
        num_devices=N_CORES,
    )

    x_d = nc.dram_tensor("x", [G * P, D], f32, kind="ExternalInput")
    w_d = nc.dram_tensor("w", [P, S], f32, kind="ExternalInput")
    o_d = nc.dram_tensor("o", [S, D], f32, kind="ExternalOutput")
    if K_o:
        xo_d = nc.dram_tensor("xo", [K_o * P, D], f32, kind="ExternalInput")
        wo_d = nc.dram_tensor("wo", [P, K_o, 8], f32, kind="ExternalInput")
        oo_d = nc.dram_tensor("oo", [8, D], f32, kind="ExternalOutput")

    with tile.TileContext(nc) as tc:
        with (
            tc.tile_pool(name="xp", bufs=6) as xpool,
            tc.tile_pool(name="wp", bufs=1) as wpool,
            tc.tile_pool(name="ac", bufs=3) as apool,
            tc.tile_pool(name="op", bufs=2) as opool,
            tc.tile_pool(name="pp", bufs=7, space="PSUM") as ppool,
            tc.tile_pool(name="ppo", bufs=1, space="PSUM") as ppool_o,
        ):
            w_tile = wpool.tile([P, S], f32)
            nc.sync.dma_start(w_tile[:], w_d.ap())

            # ---- overflow region: PE-routed via per-tile [128, 8] lhsT ----
            # Emitted AFTER slot 0 so the first guaranteed chunk's DMA (which
            # gates the DVE fold pipeline) isn't queued behind the 2 MiB
            # overflow transfer; PE has plenty of slack later in the stream.
            def emit_overflow():
                wo_tile = wpool.tile([P, K_o, 8], f32)
                nc.sync.dma_start(wo_tile[:], wo_d.ap())
                xo_v = xo_d.ap().rearrange("(p n) d -> p n d", p=P, n=K_o)
                psum_o = ppool_o.tile([8, D], f32)
                ko_chunks = [
                    (c0, min(K_o, c0 + CH)) for c0 in range(0, K_o, CH)
                ]
                done = 0
                for c0, c1 in ko_chunks:
                    xot = xpool.tile([P, CH, D], f32, tag="xov")
                    nc.sync.dma_start(xot[:, : c1 - c0, :], xo_v[:, c0:c1, :])
                    for k in range(c0, c1):
                        nc.tensor.matmul(
                            psum_o[:],
                            wo_tile[:, k, :],
                            xot[:, k - c0, :],
                            start=(done == 0),
                            stop=(done == K_o - 1),
                        )
                        done += 1
                oo_tile = opool.tile([8, D], f32)
                nc.scalar.copy(oo_tile[:], psum_o[:])
                nc.scalar.dma_start(oo_d.ap(), oo_tile[:])

            # ---- guaranteed slots: per-chunk DVE tree fold + one matmul ----
            x_ap = x_d.ap()
            off = 0
            for s in range(S):
                nt = m[s]
                w_col = w_tile[:, s : s + 1]
                x_v = x_ap[off * P : (off + nt) * P, :].rearrange(
                    "(p n) d -> p n d", p=P, n=nt
                )
                off += nt
                chunks = [(c0, min(nt, c0 + CH)) for c0 in range(0, nt, CH)]
                multi = len(chunks) > 1

                psum_t = ppool.tile([1, D], f32)
                n_mm = 1 + sum(
                    PE_K if (c1 - c0) > PE_K + 1 else 0 for c0, c1 in chunks
                )
                mm_done = 0

                def mm(rhs):
                    nonlocal mm_done
                    nc.tensor.matmul(
                        psum_t[:],
                        w_col,
                        rhs,
                        start=(mm_done == 0),
                        stop=(mm_done == n_mm - 1),
                    )
                    mm_done += 1

                acc = None
                for ci, (c0, c1) in enumerate(chunks):
                    cn = c1 - c0
                    xt = xpool.tile([P, CH, D], f32)
                    nc.sync.dma_start(xt[:, :cn, :], x_v[:, c0:c1, :])
                    pe_take = PE_K if cn > PE_K + 1 else 0
                    for k in range(cn - pe_take, cn):
                        mm(xt[:, k, :])
                    # in-place pairwise tree; odd leftovers fold into tile 0
                    w_ = cn - pe_take
                    stop_at = 2 if (multi and ci == 0 and w_ >= 2) else 1
                    while w_ > stop_at:
                        if w_ % 2:
                            nc.vector.tensor_add(
                                xt[:, 0, :], xt[:, 0, :], xt[:, w_ - 1, :]
                            )
                            w_ -= 1
                        h = w_ // 2
                        nc.vector.tensor_add(
                            xt[:, 0:h, :], xt[:, 0:h, :], xt[:, h : 2 * h, :]
                        )
                        w_ = h
                    if not multi:
                        mm(xt[:, 0, :])
                    elif ci == 0:
                        acc = apool.tile([P, D], f32)
                        if w_ == 2:
                            nc.vector.tensor_add(
                                acc[:], xt[:, 0, :], xt[:, 1, :]
                            )
                        else:
                            nc.vector.tensor_copy(acc[:], xt[:, 0, :])
                    else:
                        nc.vector.tensor_add(acc[:], acc[:], xt[:, 0, :])
                if multi:
                    mm(acc[:])

                o_tile = opool.tile([1, D], f32)
                nc.scalar.copy(o_tile[:], psum_t[:])
                nc.scalar.dma_start(o_d.ap()[s : s + 1, :], o_tile[:])

                if s == 0 and K_o:
                    emit_overflow()

    nc.compile()
    return nc


def _prepare(x, lens):
    """Pack per-core inputs. Returns (assign, key, in_maps, S)."""
    B, L, D = x.shape
    S = B // N_CORES
    assign, m, K_o, ov_tiles = _plan(lens)
    G = sum(m)
    inv = (1.0 / lens.astype(np.float64)).astype(np.float32)

    in_maps = []
    for c in range(N_CORES):
        xg = np.zeros((G * P, D), dtype=np.float32)
        off = 0
        for j in range(S):
            b = assign[c, j]
            l = int(lens[b])
            take = min(l, m[j] * P)
            xg[off * P : off * P + take] = x[b, :take]
            off += m[j]
        wc = np.broadcast_to(inv[assign[c]][None, :], (P, S))
        im = {"x": xg, "w": np.ascontiguousarray(wc)}
        if K_o:
            xo = np.zeros((K_o * P, D), dtype=np.float32)
            wo = np.zeros((P, K_o, 8), dtype=np.float32)
            ko = 0
            for j, t0, t1 in ov_tiles[c]:
                b = assign[c, j]
                l = int(lens[b])
                for t in range(t0, t1):
                    take = min(l, (t + 1) * P) - t * P
                    if take > 0:
                        xo[ko * P : ko * P + take] = x[b, t * P : t * P + take]
                    wo[:, ko, j] = inv[b]
                    ko += 1
            # device reads overflow tile n as rows {p*K_o + n}; transpose so
            # host tile n lands there with per-partition-contiguous DMA runs
            im["xo"] = np.ascontiguousarray(
                xo.reshape(K_o, P, D).transpose(1, 0, 2).reshape(K_o * P, D)
            )
            im["wo"] = wo
        in_maps.append(im)
    return assign, (S, L, D, m, K_o), in_maps


def kernel(input, length):
    from concourse.bass_interp import get_hw_module
    from concourse.bass_utils import run_bass_kernel_spmd

    x = np.asarray(input, dtype=np.float32)
    lens = np.asarray(length).astype(np.int64)
    B, L, D = x.shape
    assert B % N_CORES == 0 and L % P == 0
    S = B // N_CORES

    assign, key, in_maps, = _prepare(x, lens)
    m, K_o = key[3], key[4]

    runner = _runner_cache.get(key)
    if runner is None:
        nc = _build_program(S, D, m, K_o)
        nc.m = get_hw_module(nc.m)
        runner = nc
        _runner_cache[key] = runner

    res = run_bass_kernel_spmd(runner, in_maps, core_ids=list(range(N_CORES)))

    out = np.empty((B, D), dtype=np.float32)
    for c in range(N_CORES):
        o = res.results[c]["o"]
        if K_o:
            o = o + res.results[c]["oo"]
        out[assign[c]] = o
    return out


# revision 59
# speedup vs baseline: 1.0707x; 1.0031x over previous
"""Masked ragged-sequence mean on 8 Trainium2 NeuronCores.

out[b, d] = sum_{t < length[b]} input[b, t, d] / length[b]

Strategy (data-parallel over batch, per the problem's independence):
  - Samples are sorted by length (desc) and dealt to the 8 cores in bands
    of 8, so core slot j holds band-j samples of similar length. One SPMD
    program is compiled per length profile.
  - Per slot the program reads only the band MINIMUM tile count m_j (no
    padding waste); each core's per-sample surplus tiles are packed into a
    shared fixed-size overflow region. Guaranteed tiles are folded to
    [128, 256] by in-place pairwise trees of wide DVE adds (fp32
    tensor_tensor = 1 elem/lane/cycle) and one PE matmul with a [128, 1]
    column of 1/len reduces partitions + scales into PSUM [1, 256].
    Overflow tiles are routed on PE only: each gets a host-built [128, 8]
    lhsT whose single nonzero column (1/len in the tile's slot position)
    accumulates it into the right row of a shared [8, 256] PSUM tile.
    The host adds the overflow rows to the slot results.
  - The host zero-pads sample tails, so no on-device masking anywhere.
"""

import numpy as np

N_CORES = 8
P = 128    # SBUF partition count / token tile
CH = 11    # token tiles per DMA chunk (~1.4 MiB)
PE_K = 2   # tiles per guaranteed chunk reduced directly on PE

_runner_cache: dict = {}


def _plan(lens):
    """Band assignment + guaranteed/overflow split.

    Returns (assign[core, slot], m[slot], K_o, ov_tiles) where ov_tiles[c]
    is a list of (slot, tile_start, tile_end) per core.
    """
    B = lens.shape[0]
    S = B // N_CORES
    tiles = (lens + P - 1) // P
    order = np.argsort(-lens, kind="stable")
    assign = np.empty((N_CORES, S), dtype=np.int64)
    cum_ov = np.zeros(N_CORES, dtype=np.int64)
    m = np.empty(S, dtype=np.int64)
    # greedy per band: biggest surplus sample -> least-overflow-loaded core
    for j in range(S):
        band = order[j * N_CORES : (j + 1) * N_CORES]
        m[j] = max(1, int(tiles[band].min()))
        free = list(range(N_CORES))
        for b in sorted(band, key=lambda b: -(tiles[b] - m[j])):
            c = min(free, key=lambda c: cum_ov[c])
            assign[c, j] = b
            cum_ov[c] += tiles[b] - m[j]
            free.remove(c)
    K_o = int(cum_ov.max())
    ov_tiles = []
    for c in range(N_CORES):
        lst = []
        for j in range(S):
            t = int(tiles[assign[c, j]])
            if t > m[j]:
                lst.append((j, int(m[j]), t))
        ov_tiles.append(lst)
    return assign, tuple(int(v) for v in m), K_o, ov_tiles


def _build_program(S: int, D: int, m: tuple, K_o: int):
    import concourse.mybir as mybir
    import concourse.tile as tile
    from concourse import bacc

    f32 = mybir.dt.float32
    G = sum(m)

    nc = bacc.Bacc(
        "TRN2",
        target_bir_lowering=False,
        debug=False,
        enable_asserts=False,
        num_devices=N_CORES,
    )

    x_d = nc.dram_tensor("x", [G * P, D], f32, kind="ExternalInput")
    w_d = nc.dram_tensor("w", [P, S], f32, kind="ExternalInput")
    o_d = nc.dram_tensor("o", [S, D], f32, kind="ExternalOutput")
    if K_o:
        xo_d = nc.dram_tensor("xo", [K_o * P, D], f32, kind="ExternalInput")
        wo_d = nc.dram_tensor("wo", [P, K_o, 8], f32, kind="ExternalInput")
        oo_d = nc.dram_tensor("oo", [8, D], f32, kind="ExternalOutput")

    with tile.TileContext(nc) as tc:
        with (
            tc.tile_pool(name="xp", bufs=6) as xpool,
            tc.tile_pool(name="wp", bufs=1) as wpool,
            tc.tile_pool(name="ac", bufs=3) as apool,
            tc.tile_pool(name="op", bufs=2) as opool,
            tc.tile_pool(name="pp", bufs=7, space="PSUM") as ppool,
            tc.tile_pool(name="ppo", bufs=1, space="PSUM") as ppool_o,
        ):
            w_tile = wpool.tile([P, S], f32)
            nc.sync.dma_start(w_tile[:], w_d.ap())

            # ---- overflow region: PE-routed via per-tile [128, 8] lhsT ----
            # Emitted AFTER slot 0 so the first guaranteed chunk's DMA (which
            # gates the DVE fold pipeline) isn't queued behind the 2 MiB
            # overflow transfer; PE has plenty of slack later in the stream.
            def emit_overflow():
                wo_tile = wpool.tile([P, K_o, 8], f32)
                nc.sync.dma_start(wo_tile[:], wo_d.ap())
                xo_v = xo_d.ap().rearrange("(p n) d -> p n d", p=P, n=K_o)
                psum_o = ppool_o.tile([8, D], f32)
                ko_chunks = [
                    (c0, min(K_o, c0 + CH)) for c0 in range(0, K_o, CH)
                ]
                done = 0
                for c0, c1 in ko_chunks:
                    xot = xpool.tile([P, CH, D], f32, tag="xov")
                    nc.sync.dma_start(xot[:, : c1 - c0, :], xo_v[:, c0:c1, :])
                    for k in range(c0, c1):
                        nc.tensor.matmul(
                            psum_o[:],
                            wo_tile[:, k, :],
                            xot[:, k - c0, :],
                            start=(done == 0),
                            stop=(done == K_o - 1),
                        )
                        done += 1
                oo_tile = opool.tile([8, D], f32)
                nc.scalar.copy(oo_tile[:], psum_o[:])
                nc.scalar.dma_start(oo_d.ap(), oo_tile[:])

            # ---- guaranteed slots: per-chunk DVE tree fold + one matmul ----
            x_ap = x_d.ap()
            off = 0
            for s in range(S):
                nt = m[s]
                w_col = w_tile[:, s : s + 1]
                x_v = x_ap[off * P : (off + nt) * P, :].rearrange(
                    "(p n) d -> p n d", p=P, n=nt
                )
                off += nt
                chunks = [(c0, min(nt, c0 + CH)) for c0 in range(0, nt, CH)]
                multi = len(chunks) > 1

                psum_t = ppool.tile([1, D], f32)
                n_mm = 1 + sum(
                    PE_K if (c1 - c0) > PE_K + 1 else 0 for c0, c1 in chunks
                )
                mm_done = 0

                def mm(rhs):
                    nonlocal mm_done
                    nc.tensor.matmul(
                        psum_t[:],
                        w_col,
                        rhs,
                        start=(mm_done == 0),
                        stop=(mm_done == n_mm - 1),
                    )
                    mm_done += 1

                acc = None
                for ci, (c0, c1) in enumerate(chunks):
                    cn = c1 - c0
                    xt = xpool.tile([P, CH, D], f32)
                    nc.sync.dma_start(xt[:, :cn, :], x_v[:, c0:c1, :])
                    pe_take = PE_K if cn > PE_K + 1 else 0
                    for k in range(cn - pe_take, cn):
                        mm(xt[:, k, :])
                    # in-place pairwise tree; odd leftovers fold into tile 0
                    w_ = cn - pe_take
                    stop_at = 2 if (multi and ci == 0 and w_ >= 2) else 1
                    while w_ > stop_at:
                        if w_ % 2:
                            nc.vector.tensor_add(
                                xt[:, 0, :], xt[:, 0, :], xt[:, w_ - 1, :]
                            )
                            w_ -= 1
                        h = w_ // 2
                        nc.vector.tensor_add(
                            xt[:, 0:h, :], xt[:, 0:h, :], xt[:, h : 2 * h, :]
                        )
                        w_ = h
                    if not multi:
                        mm(xt[:, 0, :])
                    elif ci == 0:
                        acc = apool.tile([P, D], f32)
                        if w_ == 2:
                            nc.vector.tensor_add(
                                acc[:], xt[:, 0, :], xt[:, 1, :]
                            )
                        else:
                            nc.vector.tensor_copy(acc[:], xt[:, 0, :])
                    else:
                        nc.vector.tensor_add(acc[:], acc[:], xt[:, 0, :])
                if multi:
                    mm(acc[:])

                o_tile = opool.tile([1, D], f32)
                nc.scalar.copy(o_tile[:], psum_t[:])
                nc.scalar.dma_start(o_d.ap()[s : s + 1, :], o_tile[:])

                if s == 0 and K_o:
                    emit_overflow()

    nc.compile()
    return nc


def _prepare(x, lens):
    """Pack per-core inputs. Returns (assign, key, in_maps, S)."""
    B, L, D = x.shape
    S = B // N_CORES
    assign, m, K_o, ov_tiles = _plan(lens)
    G = sum(m)
    inv = (1.0 / lens.astype(np.float64)).astype(np.float32)

    in_maps = []
    for c in range(N_CORES):
        xg = np.zeros((G * P, D), dtype=np.float32)
        off = 0
        for j in range(S):
            b = assign[c, j]
            l = int(lens[b])
            take = min(l, m[j] * P)
            xg[off * P : off * P + take] = x[b, :take]
            off += m[j]
        wc = np.broadcast_to(inv[assign[c]][None, :], (P, S))
        im = {"x": xg, "w": np.ascontiguousarray(wc)}
        if K_o:
            xo = np.zeros((K_o * P, D), dtype=np.float32)
            wo = np.zeros((P, K_o, 8), dtype=np.float32)
            ko = 0
            for j, t0, t1 in ov_tiles[c]:
                b = assign[c, j]
                l = int(lens[b])
                for t in range(t0, t1):
                    take = min(l, (t + 1) * P) - t * P
                    if take > 0:
                        xo[ko * P : ko * P + take] = x[b, t * P : t * P + take]
                    wo[:, ko, j] = inv[b]
                    ko += 1
            # device reads overflow tile n as rows {p*K_o + n}; transpose so
            # host tile n lands there with per-partition-contiguous DMA runs
            im["xo"] = np.ascontiguousarray(
                xo.reshape(K_o, P, D).transpose(1, 0, 2).reshape(K_o * P, D)
            )
            im["wo"] = wo
        in_maps.append(im)
    return assign, (S, L, D, m, K_o), in_maps


def kernel(input, length):
    from concourse.bass_interp import get_hw_module
    from concourse.bass_utils import run_bass_kernel_spmd

    x = np.asarray(input, dtype=np.float32)
    lens = np.asarray(length).astype(np.int64)
    B, L, D = x.shape
    assert B % N_CORES == 0 and L % P == 0
    S = B // N_CORES

    assign, key, in_maps, = _prepare(x, lens)
    m, K_o = key[3], key[4]

    runner = _runner_cache.get(key)
    if runner is None:
        nc = _build_program(S, D, m, K_o)
        nc.m = get_hw_module(nc.m)
        runner = nc
        _runner_cache[key] = runner

    res = run_bass_kernel_spmd(runner, in_maps, core_ids=list(range(N_CORES)))

    out = np.empty((B, D), dtype=np.float32)
    for c in range(N_CORES):
        o = res.results[c]["o"]
        if K_o:
            o = o + res.results[c]["oo"]
        out[assign[c]] = o
    return out
